# revision 38
# baseline (speedup 1.0000x reference)
"""ArcticDecoderLayer on 8 TRN2 NeuronCores — v2.

Sharding (token-parallel + expert-parallel):
  - tokens: zigzag blocks (core i owns 128-token blocks {i, 15-i});
    attention / wo / residual MLP token-parallel, weights replicated.
  - MoE: expert-parallel (2 experts/core), host-side routing + gather.
  - One fp8 KV AllGather.

v2 design:
  - MoE expert GEMMs in fp8e4 DoubleRow (256-deep contraction per matmul);
    qkv/wo/w13/w2 in fp16 (same PE speed as bf16, 4x less rounding --
    attention scores amplify operand rounding by ~sqrt(256)).
  - All weights host-prepacked to the exact SBUF stream layout (one
    contiguous run per partition per DMA; ~10x fewer descriptors).
  - Attention: head-pairs share the kv lhsT (uniform N=512 score matmuls,
    16 slots); batched Exp ACTs (N=1024, no bias); ALL causal masking
    (block + diagonal tri) via one resident 0/1 mask multiply; denominator
    from the SAME fp16 pdn as PV so quantization cancels in the ratio
    (one N=512 matmul per slot, fast-approx reciprocal).
  - silu via tanh (same ACT table set as Exp -> no table thrash);
    0.5 factor folded into w2/w2s weights host-side.
  - KV AllGather ships fp16 K+V as raw bytes; unpack lands in 2KB-run
    descriptors; MoE sweeps are thunks interleaved into the attention
    and w13 phases to keep the PE warm.
"""
import numpy as np
import ml_dtypes

import concourse.bacc as bacc
import concourse.tile as tile
import concourse.mybir as mybir
from concourse.bass_utils import run_bass_kernel_spmd

F32 = mybir.dt.float32
BF16 = mybir.dt.bfloat16
H16 = mybir.dt.float16
F8 = mybir.dt.float8e4
AF = mybir.ActivationFunctionType
DR = mybir.MatmulPerfMode.DoubleRow

H = 2048
NH = 16
NKV = 4
HD = 128
HALF = 64
I = 1024
E = 16
TOPK = 2
T = 2048
EPS = 1e-5
THETA = 10000.0
NC_ = 8
BLK = 128
NBLK = 16
TPC = 256  # tokens per core
EPC = 2  # experts per core
SCALE = HD ** -0.5
KQ = H // BLK  # 16
KK = KQ // 2  # 8 contraction pairs
MI = I // BLK  # 8

TRACE = False
DEBUG_TAPS = False
LAST_RESULT = None
_CACHE = {}

bf = lambda a: np.ascontiguousarray(np.asarray(a).astype(ml_dtypes.bfloat16))
h16 = lambda a: np.ascontiguousarray(np.asarray(a).astype(np.float16))
f32 = lambda a: np.ascontiguousarray(a, dtype=np.float32)
f8 = lambda a: np.ascontiguousarray(
    np.clip(np.asarray(a, dtype=np.float32), -240, 240).astype(ml_dtypes.float8_e4m3))

KG2 = 4  # contraction-pair chunks per weight-stream DMA
SW = 8   # m-chunks per sweep for TN=256 GEMMs
MSW = 2  # m-chunks per sweep for MoE GEMMs


def _pack_dr(W, sweep):
    """W [K, M] -> fp8 [128, (M//(128*sweep)) * (K//256) * sweep * 2 * 128]
    laid out [p, s, kk, jl, two, m] so any (sweep, kk-range) slice is one
    contiguous run per partition."""
    K, M = W.shape
    k2 = K // 256
    S = M // (BLK * sweep)
    W6 = W.reshape(k2, 2, BLK, S, sweep, BLK)      # kk, a, p, s, jl, m
    W6 = W6.transpose(2, 3, 0, 4, 1, 5)            # p, s, kk, jl, a, m
    return f8(W6.reshape(BLK, -1))


def _pack_bf(W, sweep):
    """W [K, M] -> bf16 [128, (M//(128*sweep)) * (K//128) * sweep * 128]
    laid out [p, s, k, jl, m]."""
    K, M = W.shape
    kc = K // BLK
    S = M // (BLK * sweep)
    W5 = W.reshape(kc, BLK, S, sweep, BLK)         # k, p, s, jl, m
    W5 = W5.transpose(1, 2, 0, 3, 4)               # p, s, k, jl, m
    return h16(W5.reshape(BLK, -1))


def _interleave_cols(w, half):
    # [rows, 2*half] -> column chunks reordered so chunk 2p=g_p, 2p+1=u_p
    rows = w.shape[0]
    g = w[:, :half].reshape(rows, half // BLK, BLK)
    u = w[:, half:].reshape(rows, half // BLK, BLK)
    out = np.empty((rows, 2 * (half // BLK), BLK), w.dtype)
    out[:, 0::2] = g
    out[:, 1::2] = u
    return out.reshape(rows, 2 * half // BLK * BLK)


def _build(cap):
    nc = bacc.Bacc("TRN2", target_bir_lowering=False, debug=False, num_devices=NC_)
    capk = cap // 2  # not used for contraction; cap is token width

    din = lambda name, shape, dt=F8: nc.dram_tensor(name, shape, dt, kind="ExternalInput")
    xnb_d = din("xnb", [BLK, KQ * TPC], H16)      # normalized x^T (qkv rhs), bf16
    xr_d = din("xr", [BLK, KQ * TPC], H16)        # raw x^T (residual), bf16
    cos_d = din("cos2", [HALF, 2 * TPC], F32)
    sin_d = din("sin2", [HALF, 2 * TPC], F32)
    mask_d = din("mask", [BLK, NBLK * TPC])  # [key_p, slot, q] 0/1 (incl. tri)
    ident_d = din("ident", [BLK, BLK], H16)
    wqkv_d = din("wqkv", [BLK, 3 * KQ * SW * BLK], H16)    # bf16 packed, 24 m-chunks
    wo_d = din("wo", [BLK, 2 * KQ * SW * BLK], H16)        # bf16, 16 m-chunks
    w13_d = din("w13", [BLK, 4 * KQ * SW * BLK], H16)       # 32 m-chunks (g/u interleaved)
    w2_d = din("w2", [BLK, 2 * KQ * SW * BLK], H16)         # 16 m-chunks
    ws_d = din("wsT", [EPC, BLK, 8 * KK * MSW * 2 * BLK])   # per expert 16 m-chunks
    w2s_d = din("w2sT", [EPC, BLK, 8 * (KK // 2) * MSW * 2 * BLK])  # 16 m, contraction I
    xg_d = din("xgT", [EPC, BLK, KQ * cap])
    ew_d = din("ew", [EPC, BLK, cap], BF16)

    res_out_d = nc.dram_tensor("res_out", [BLK, KQ * TPC], F32, kind="ExternalOutput")
    moe_out_d = nc.dram_tensor("moe_out", [EPC, BLK, KQ * cap], F32, kind="ExternalOutput")
    taps = {}
    if DEBUG_TAPS:
        for nm, w, dt in [("q", NH * TPC, H16), ("k", NKV * TPC, H16),
                          ("v8", 2 * NKV * HD, H16), ("kag", NC_ * NKV * 2 * BLK, H16),
                          ("vag", NC_ * 2 * NKV * BLK, H16),
                          ("pdn0", 2 * NBLK * TPC, H16), ("attn8", NH * TPC, H16),
                          ("resid", KQ * TPC, H16), ("h2", KQ * TPC, H16),
                          ("gu", KQ * TPC, H16), ("hm", EPC * MI * cap, F8),
                          ("apv0", 2 * TPC, F32), ("den0", 2 * TPC, F32)]:
            taps[nm] = nc.dram_tensor("tap_" + nm, [BLK, w], dt, kind="ExternalOutput")

    with tile.TileContext(nc) as tc:
        with (
            tc.tile_pool(name="res", bufs=1) as res,
            tc.tile_pool(name="stream", bufs=1 if DEBUG_TAPS else 2) as stream,
            tc.tile_pool(name="small", bufs=2) as small,
            tc.tile_pool(name="outp", bufs=2) as outp,
            tc.tile_pool(name="sps", bufs=2, space="PSUM") as sps,    # 2x [128,1024] = 4 banks
            tc.tile_pool(name="apv", bufs=1, space="PSUM") as apvp,   # 1 bank
            tc.tile_pool(name="dpsp", bufs=1, space="PSUM") as dpsp,  # 1 bank
            tc.tile_pool(name="macc", bufs=2, space="PSUM") as macc,  # 2 banks
            tc.tile_pool(name="dram", bufs=1, space="DRAM") as dram,
        ):
            eng_rr = [nc.sync, nc.scalar]

            # ---------------- resident loads ----------------
            xnb_sb = res.tile([BLK, KQ * TPC], H16, tag="xnb")
            nc.sync.dma_start(xnb_sb[:], xnb_d[:])
            cos2_sb = res.tile([HALF, 2 * TPC], F32, tag="cos")
            sin2_sb = res.tile([HALF, 2 * TPC], F32, tag="sin")
            nc.sync.dma_start(cos2_sb[:], cos_d[:])
            nc.sync.dma_start(sin2_sb[:], sin_d[:])
            ident_sb = res.tile([BLK, BLK], H16, tag="ident")
            nc.sync.dma_start(ident_sb[:], ident_d[:])
            mask_sb = res.tile([BLK, NBLK * TPC], F8, tag="mask")
            nc.scalar.dma_start(mask_sb[:], mask_d[:])
            xg_sb = res.tile([BLK, EPC * KQ * cap], F8, tag="xg")
            ew_sb = res.tile([BLK, EPC * cap], BF16, tag="ew")
            for e in range(EPC):
                nc.scalar.dma_start(xg_sb[:, e * KQ * cap:(e + 1) * KQ * cap], xg_d[e])
                nc.scalar.dma_start(ew_sb[:, e * cap:(e + 1) * cap], ew_d[e])
            xr_sb = res.tile([BLK, KQ * TPC], H16, tag="xr")
            nc.scalar.dma_start(xr_sb[:], xr_d[:])

            ones8_sb = res.tile([BLK, 32], F8, tag="ones8")
            nc.vector.memset(ones8_sb[:], 1.0)
            ones_row = res.tile([1, BLK], H16, tag="onesr")
            nc.vector.memset(ones_row[:], 1.0)
            ones_cb = res.tile([BLK, 1], H16, tag="onescb")
            nc.vector.memset(ones_cb[:], 1.0)

            q_sb = res.tile([BLK, NH * TPC], H16, tag="q")
            k8_sb = res.tile([BLK, NKV * TPC], H16, tag="k8")
            v8_sb = res.tile([BLK, 2 * NKV * HD], H16, tag="v8")
            kag_sb = res.tile([BLK, NC_ * NKV * 2 * BLK], H16, tag="kag")
            vag_sb = res.tile([BLK, NC_ * 2 * NKV * BLK], H16, tag="vag")
            attn8_sb = res.tile([BLK, NH * TPC], H16, tag="attn8")
            resid_sb = res.tile([BLK, KQ * TPC], H16, tag="resid")
            h2_sb = res.tile([BLK, KQ * TPC], H16, tag="h2")
            gu_sb = res.tile([BLK, KQ * TPC], H16, tag="gu")
            hm_sb = res.tile([BLK, EPC * MI * cap], F8, tag="hm")

            # ============ streamed GEMM sweep ============
            # dr=True: fp8 DoubleRow, weights [p, s, kk, jl, two, m],
            #   rhs_fn(kk) -> [128, 2, tn]; kkcnt = contraction pairs.
            # dr=False: bf16, weights [p, s, k, jl, m], rhs_fn(k) -> [128, tn];
            #   kkcnt = contraction chunks.
            def gemm(w_flat, mcnt, kkcnt, rhs_fn, tn, consume, sweep, tag,
                     kg2=KG2, sweep_starts=None, tile_w=1024, dr=True):
                nsweep = mcnt // sweep
                starts = sweep_starts if sweep_starts is not None else range(nsweep)
                per_tile = tile_w // tn  # m-chunks per psum tile
                wstep = (2 if dr else 1) * BLK  # weight cols per (k, j)
                for s in starts:
                    ntile = (sweep + per_tile - 1) // per_tile
                    if tag == "macct":
                        pts = [macc.tile([BLK, tile_w], F32, tag=tag, name=f"pt{j}")
                               for j in range(ntile)]
                    else:
                        pts = [sps.tile([BLK, tile_w], F32, tag=tag, name=f"pt{j}")
                               for j in range(ntile)]
                    paps = [pts[j // per_tile][:, (j % per_tile) * tn:(j % per_tile + 1) * tn]
                            for j in range(sweep)]
                    for kg0 in range(0, kkcnt, kg2):
                        kgn = min(kg2, kkcnt - kg0)
                        wt = stream.tile([BLK, KG2 * SW * (2 if dr else 1) * BLK],
                                         F8 if dr else H16, tag="wt")
                        off = (s * kkcnt + kg0) * sweep * wstep
                        eng_rr[(kg0 // kg2) % 2].dma_start(
                            wt[:, :kgn * sweep * wstep],
                            w_flat[:, off: off + kgn * sweep * wstep])
                        for kl in range(kgn):
                            kk = kg0 + kl
                            for j in range(sweep):
                                # bank = 512 f32; chunks sharing a bank pair their
                                # start/stop (start clears the whole bank)
                                per_bank = min(per_tile, max(1, 512 // tn))
                                jb = (j % per_tile) % per_bank
                                first = jb == 0
                                last = (jb == per_bank - 1) or (j == sweep - 1)
                                wap = wt[:, (kl * sweep + j) * wstep:(kl * sweep + j + 1) * wstep]
                                nc.tensor.matmul(
                                    paps[j],
                                    wap.rearrange("p (a m) -> p a m", a=2) if dr else wap,
                                    rhs_fn(kk),
                                    start=(kk == 0 and first),
                                    stop=(kk == kkcnt - 1 and last),
                                    perf_mode=DR if dr else None)
                    consume(s, sweep, paps, pts)

            # ---------------- QKV projection ----------------
            def rope_pair(pt2, dst, col0):
                # pt2 [128, 512] psum (two head-chunks side by side)
                t1 = small.tile([HALF, 2 * TPC], F32, tag="r1")
                t2 = small.tile([HALF, 2 * TPC], F32, tag="r2")
                nc.vector.tensor_mul(t1[:], pt2[0:HALF, :], cos2_sb[:])
                nc.vector.tensor_mul(t2[:], pt2[HALF:BLK, :], sin2_sb[:])
                nc.vector.tensor_sub(dst[0:HALF, col0:col0 + 2 * TPC], t1[:], t2[:])
                t3 = small.tile([HALF, 2 * TPC], F32, tag="r1")
                t4 = small.tile([HALF, 2 * TPC], F32, tag="r2")
                nc.vector.tensor_mul(t3[:], pt2[HALF:BLK, :], cos2_sb[:])
                nc.vector.tensor_mul(t4[:], pt2[0:HALF, :], sin2_sb[:])
                nc.vector.tensor_add(dst[HALF:BLK, col0:col0 + 2 * TPC], t3[:], t4[:])

            def qkv_consume(s, sweep, paps, pts):
                for jt, pt in enumerate(pts):
                    for half_t in range(2):
                        m = s * SW + jt * 4 + half_t * 2
                        pt2 = pt[:, half_t * 512:(half_t + 1) * 512]
                        if m < NH:
                            rope_pair(pt2, q_sb, m * TPC)
                        elif m < NH + NKV:
                            rope_pair(pt2, k8_sb, (m - NH) * TPC)
                        else:
                            for hj in range(2):
                                kvh = m + hj - NH - NKV
                                ps = pt[:, (half_t * 2 + hj) * TPC:(half_t * 2 + hj + 1) * TPC]
                                vtmp = small.tile([BLK, TPC], H16, tag="vtmp")
                                nc.vector.tensor_copy(vtmp[:], ps)
                                for tb in range(2):
                                    ptt = macc.tile([BLK, 1024], H16, tag="macct")
                                    nc.tensor.transpose(ptt[:, 0:BLK], vtmp[:, tb * BLK:(tb + 1) * BLK], ident_sb[:])
                                    nc.vector.tensor_copy(
                                        v8_sb[:, (tb * NKV + kvh) * BLK:(tb * NKV + kvh + 1) * BLK],
                                        ptt[:, 0:BLK])

            qkv_rhs = lambda k: xnb_sb[:, k * TPC:(k + 1) * TPC]
            # KV sweep first so the AllGather can launch early
            gemm(wqkv_d, 3 * SW, KQ, qkv_rhs, TPC, qkv_consume, SW, "acct",
                 sweep_starts=[2], dr=False)

            # ---------------- KV AllGather (K fp8 + V bf16, raw bytes) ----------------
            KSZ = NKV * BLK * TPC  # K bytes; V is 2*KSZ bytes
            U8 = mybir.dt.uint8
            kv_local = dram.tile([4 * KSZ], U8)
            kv_ag = dram.tile([NC_, 4 * KSZ], U8, addr_space="Shared")
            # K bf16: [d, (h sub t)] flat copy
            nc.sync.dma_start(kv_local[0:2 * KSZ].rearrange("(d x) -> d x", d=BLK),
                              k8_sb[:].bitcast(U8))
            # V bf16: [t, (sub h d)] flat copy
            nc.sync.dma_start(kv_local[2 * KSZ:4 * KSZ].rearrange("(t x) -> t x", t=BLK),
                              v8_sb[:].bitcast(U8))
            nc.gpsimd.collective_compute(
                "AllGather", mybir.AluOpType.bypass,
                replica_groups=[list(range(NC_))],
                ins=[kv_local[:]], outs=[kv_ag[:]])
            # remaining qkv sweeps (q heads) overlap the collective
            gemm(wqkv_d, 3 * SW, KQ, qkv_rhs, TPC, qkv_consume, SW, "acct",
                 sweep_starts=[0, 1], dr=False)
            # unpack: kag [d, (c h sub t)], vag [t, (c sub h d)]
            nc.sync.dma_start(
                kag_sb[:].bitcast(U8).rearrange("d (c x) -> d c x", c=NC_),
                kv_ag[:, 0:2 * KSZ].rearrange("c (d x) -> d c x", d=BLK))
            nc.scalar.dma_start(
                vag_sb[:].bitcast(U8).rearrange("t (c x) -> t c x", c=NC_),
                kv_ag[:, 2 * KSZ:4 * KSZ].rearrange("c (t x) -> t c x", t=BLK))

            # ------- MoE sweeps (thunks) interleaved with attention -------
            moe_thunks = []
            for e in range(EPC):
                gu_rhs = lambda kk, e=e: xg_sb[:, (e * KQ + 2 * kk) * cap:(e * KQ + 2 * kk + 2) * cap] \
                    .rearrange("p (a t) -> p a t", a=2)

                def gu_consume(s, sweep, paps, pts, e=e):
                    # sweep=2: chunks (2s, 2s+1) = (g_p, u_p) pair, p = s
                    gps, ups = paps[0], paps[1]
                    sg = small.tile([BLK, cap], BF16, tag="sg")
                    nc.scalar.activation(sg[:], gps, AF.Tanh, scale=0.5)
                    u8 = small.tile([BLK, cap], BF16, tag="u8")
                    nc.vector.tensor_copy(u8[:], ups)
                    t1 = small.tile([BLK, cap], BF16, tag="sgt1")
                    nc.vector.tensor_mul(t1[:], gps, u8[:])
                    t2 = small.tile([BLK, cap], BF16, tag="sgt2")
                    nc.vector.tensor_mul(t2[:], t1[:], sg[:])
                    nc.vector.tensor_add(
                        hm_sb[:, (e * MI + s) * cap:(e * MI + s + 1) * cap],
                        t1[:], t2[:])

                def w2s_consume(s, sweep, paps, pts, e=e):
                    for jl, ps in enumerate(paps):
                        m = s * MSW + jl
                        mo = outp.tile([BLK, cap], F32, tag="mo")
                        nc.vector.tensor_mul(mo[:], ps, ew_sb[:, e * cap:(e + 1) * cap])
                        nc.scalar.dma_start(
                            moe_out_d[e, :, m * cap:(m + 1) * cap], mo[:])

                w2s_rhs = lambda kk, e=e: hm_sb[:, (e * MI + 2 * kk) * cap:(e * MI + 2 * kk + 2) * cap] \
                    .rearrange("p (a t) -> p a t", a=2)

                for s in range(MI):  # 8 gu sweeps (one g/u pair each)
                    moe_thunks.append(lambda s=s, e=e, r=gu_rhs, c=gu_consume: gemm(
                        ws_d[e], 2 * MI, KK, r, cap, c, MSW, "macct",
                        sweep_starts=[s], tile_w=cap))
                for s in range(KQ // MSW):  # 8 w2s sweeps
                    moe_thunks.append(lambda s=s, e=e, r=w2s_rhs, c=w2s_consume: gemm(
                        w2s_d[e], KQ, KK // 2, r, cap, c, MSW, "macct",
                        sweep_starts=[s], tile_w=cap))

            # ---------------- attention (head pairs) ----------------
            def attention_scores(g):
                h0 = 2 * g
                qv = q_sb[:, h0 * TPC:(h0 + 2) * TPC]  # [d, 512]
                kvh = h0 // (NH // NKV)
                pdn = small.tile([BLK, 2 * NBLK * TPC], H16, tag="pdn",
                                 name=f"pdn{g}")  # [k, h2, slot, q] for PV
                for so in range(0, NBLK, 2):  # slot pairs -> one psum tile
                    spt = sps.tile([BLK, 1024], F32, tag="acct", name="spt")
                    for sl in (so, so + 1):
                        c, sub = sl // 2, sl % 2
                        kap = kag_sb[:, ((c * NKV + kvh) * 2 + sub) * BLK:
                                     ((c * NKV + kvh) * 2 + sub + 1) * BLK]
                        nc.tensor.matmul(spt[:, (sl - so) * 512:(sl - so + 1) * 512],
                                         kap, qv, start=True, stop=True)
                    # exp straight into pdn [k, h2, slot, q] (out AP in (s,h,q) order)
                    nc.scalar.activation(
                        pdn[:].rearrange("p (h s q) -> p s h q", h=2, s=NBLK)[:, so:so + 2, :, :],
                        spt[:], AF.Exp, scale=SCALE)
                    # mask in place
                    for hh in range(2):
                        nc.vector.tensor_mul(
                            pdn[:].rearrange("p (h s q) -> p h s q", h=2, s=NBLK)[:, hh, so:so + 2, :],
                            pdn[:].rearrange("p (h s q) -> p h s q", h=2, s=NBLK)[:, hh, so:so + 2, :],
                            mask_sb[:].rearrange("p (s q) -> p s q", s=NBLK)[:, so:so + 2, :])
                return pdn

            def attention_pv(g, pdn):
                h0 = 2 * g
                kvh = h0 // (NH // NKV)
                apv = apvp.tile([BLK, 2 * TPC], F32, tag="apvt")
                dps = dpsp.tile([BLK, 2 * TPC], F32, tag="dpst")
                for sl in range(NBLK):
                    c, sub = sl // 2, sl % 2
                    vap = vag_sb[:, ((c * 2 + sub) * NKV + kvh) * BLK:
                                 ((c * 2 + sub) * NKV + kvh + 1) * BLK]
                    nc.tensor.matmul(
                        apv[:], vap,
                        pdn[:].rearrange("p (h s q) -> p h s q", h=2, s=NBLK)[:, :, sl, :],
                        start=(sl == 0), stop=(sl == NBLK - 1))
                for sl in range(NBLK):
                    # den from the SAME fp16 pdn as PV so quantization cancels
                    # in the ratio; one MM covers both heads (free = (h, q)).
                    nc.tensor.matmul(
                        dps[0:1, :],
                        ones_cb[:],
                        pdn[:].rearrange("p (h s q) -> p h s q", h=2, s=NBLK)[:, :, sl, :],
                        start=(sl == 0), stop=(sl == NBLK - 1))
                if DEBUG_TAPS and g == 0:
                    at = small.tile([BLK, 2 * TPC], F32, tag="apvtap")
                    nc.vector.tensor_copy(at[:], apv[:])
                    nc.sync.dma_start(taps["apv0"].ap()[0:BLK, :], at[:])
                    dt_ = small.tile([1, 2 * TPC], F32, tag="dentap")
                    nc.vector.tensor_copy(dt_[:], dps[0:1, :])
                    nc.sync.dma_start(taps["den0"].ap()[0:1, :], dt_[:])
                # normalize: rec -> broadcast -> attn8
                rec32 = small.tile([1, 2 * TPC], F32, tag="rec32")
                nc.vector.reciprocal_approx_fast(rec32[:], dps[0:1, :])
                rec = small.tile([1, 2 * TPC], H16, tag="rec")
                nc.vector.tensor_copy(rec[:], rec32[:])
                bct = sps.tile([BLK, 1024], F32, tag="acct", name="bct")
                nc.tensor.matmul(bct[:, 0:512], ones_row[:], rec[:], start=True, stop=True)
                bcs = small.tile([BLK, 2 * TPC], H16, tag="bcs")
                nc.vector.tensor_copy(bcs[:], bct[:, 0:512])
                nc.vector.tensor_mul(attn8_sb[:, h0 * TPC:(h0 + 2) * TPC], apv[:], bcs[:])

            # front-load MoE sweeps to cover the AllGather; rest interleave.
            # scores(g) issue before pv(g-1) so the PV never waits on exp/mask.
            nfront, ntail = 3, 5
            for th in moe_thunks[:nfront]:
                th()
            rest = moe_thunks[nfront:len(moe_thunks) - ntail]
            tail_thunks = moe_thunks[len(moe_thunks) - ntail:]
            ri = 0
            for g in range(NH // 2):
                for _ in range(3):
                    if ri < len(rest):
                        rest[ri]()
                        ri += 1
                pdn_cur = attention_scores(g)
                if DEBUG_TAPS and g == 0:
                    nc.sync.dma_start(taps["pdn0"].ap(), pdn_cur[:])
                attention_pv(g, pdn_cur)
            while ri < len(rest):
                rest[ri]()
                ri += 1

            # ---------------- wo + residual ----------------
            def wo_consume(s, sweep, paps, pts):
                for jt, pt in enumerate(pts):
                    m0 = s * SW + jt * 4
                    for q in range(2):
                        nc.vector.tensor_add(
                            resid_sb[:, (m0 + 2 * q) * TPC:(m0 + 2 * q + 2) * TPC],
                            pt[:, q * 512:(q + 1) * 512],
                            xr_sb[:, (m0 + 2 * q) * TPC:(m0 + 2 * q + 2) * TPC])

            wo_rhs = lambda k: attn8_sb[:, k * TPC:(k + 1) * TPC]
            gemm(wo_d, 2 * SW, KQ, wo_rhs, TPC, wo_consume, SW, "acct", dr=False)

            # ---------------- residual MLP norm scale ----------------
            ssq = apvp.tile([BLK, 2 * TPC], F32, tag="apvt")
            for k in range(KQ):
                sq = small.tile([BLK, TPC], H16, tag="sq")
                nc.vector.tensor_mul(sq[:], resid_sb[:, k * TPC:(k + 1) * TPC],
                                     resid_sb[:, k * TPC:(k + 1) * TPC])
                nc.tensor.matmul(ssq[0:1, 0:TPC], ones_cb[:], sq[:],
                                 start=(k == 0), stop=(k == KQ - 1))
            vt = small.tile([1, TPC], F32, tag="vt")
            nc.vector.tensor_scalar(vt[:], ssq[0:1, 0:TPC], 1.0 / H, EPS,
                                    mybir.AluOpType.mult, mybir.AluOpType.add)
            st = small.tile([1, TPC], F32, tag="vt2")
            nc.scalar.activation(st[:], vt[:], AF.Sqrt)
            sr = small.tile([1, TPC], H16, tag="vt3")
            with nc.allow_low_precision(reason="rmsnorm rsqrt in bf16"):
                nc.vector.reciprocal(sr[:], st[:])
            s2p = dpsp.tile([BLK, 2 * TPC], F32, tag="dpst")
            nc.tensor.matmul(s2p[:, 0:TPC], ones_row[:], sr[:], start=True, stop=True)
            s2s = small.tile([BLK, TPC], F32, tag="s2s")
            nc.vector.tensor_copy(s2s[:], s2p[:, 0:TPC])
            for k in range(KQ):
                nc.vector.tensor_mul(h2_sb[:, k * TPC:(k + 1) * TPC],
                                     resid_sb[:, k * TPC:(k + 1) * TPC], s2s[:])

            # ---------------- w13 (interleaved g/u) + silu ----------------
            def w13_consume(s, sweep, paps, pts):
                for jt, pt in enumerate(pts):
                    for half_t in range(2):
                        p = (s * SW + jt * 4) // 2 + half_t
                        gps = pt[:, half_t * 512:half_t * 512 + TPC]
                        ups = pt[:, half_t * 512 + TPC:(half_t + 1) * 512]
                        sg = small.tile([BLK, TPC], BF16, tag="sg13")
                        nc.scalar.activation(sg[:], gps, AF.Tanh, scale=0.5)
                        u8 = small.tile([BLK, TPC], BF16, tag="u813")
                        nc.vector.tensor_copy(u8[:], ups)
                        t1 = small.tile([BLK, TPC], BF16, tag="t113")
                        nc.vector.tensor_mul(t1[:], gps, u8[:])
                        t2 = small.tile([BLK, TPC], BF16, tag="t213")
                        nc.vector.tensor_mul(t2[:], t1[:], sg[:])
                        nc.vector.tensor_add(gu_sb[:, p * TPC:(p + 1) * TPC],
                                             t1[:], t2[:])

            w13_rhs = lambda k: h2_sb[:, k * TPC:(k + 1) * TPC]
            for i, s in enumerate(range(4)):
                if i < len(tail_thunks):
                    tail_thunks[i]()
                gemm(w13_d, 4 * SW, KQ, w13_rhs, TPC, w13_consume, SW, "acct",
                     sweep_starts=[s], dr=False)
            for i in range(4, len(tail_thunks)):
                tail_thunks[i]()

            # ---------------- w2 + final out ----------------
            def w2_consume(s, sweep, paps, pts):
                for jt, pt in enumerate(pts):
                    m0 = s * SW + jt * 4
                    for q in range(2):
                        fo = outp.tile([BLK, 512], F32, tag="fo")
                        nc.vector.tensor_add(
                            fo[:], pt[:, q * 512:(q + 1) * 512],
                            resid_sb[:, (m0 + 2 * q) * TPC:(m0 + 2 * q + 2) * TPC])
                        nc.sync.dma_start(
                            res_out_d[:, (m0 + 2 * q) * TPC:(m0 + 2 * q + 2) * TPC],
                            fo[:])

            w2_rhs = lambda k: gu_sb[:, k * TPC:(k + 1) * TPC]
            gemm(w2_d, 2 * SW, KQ, w2_rhs, TPC, w2_consume, SW, "acct", dr=False)

            if DEBUG_TAPS:
                for nm, sb in [("q", q_sb), ("k", k8_sb), ("v8", v8_sb),
                               ("kag", kag_sb), ("vag", vag_sb),
                               ("attn8", attn8_sb), ("resid", resid_sb),
                               ("h2", h2_sb), ("gu", gu_sb), ("hm", hm_sb)]:
                    nc.sync.dma_start(taps[nm].ap(), sb[:])

    nc.compile()
    return nc


def kernel(**inputs):
    global LAST_RESULT
    hidden = f32(inputs["hidden_states"])
    positions = np.asarray(inputs["positions"]).astype(np.float32)
    ln_in_w = f32(inputs["ln_in_w"])
    ln_post_w = f32(inputs["ln_post_w"])
    ln_res_w = f32(inputs["ln_res_w"])
    wqkv = f32(inputs["wqkv"])
    wo = f32(inputs["wo"])
    res_w13 = f32(inputs["res_w13"])
    res_w2 = f32(inputs["res_w2"])
    gate_w = f32(inputs["gate_w"])
    ws = f32(inputs["ws"])
    w2s = f32(inputs["w2s"])

    # ---- host prep (routing + sharding) ----
    s = 1.0 / np.sqrt(np.mean(hidden * hidden, axis=1) + EPS)  # [T]
    x_norm = hidden * s[:, None]

    logits = (x_norm * ln_post_w) @ gate_w
    pr = np.exp(logits - logits.max(-1, keepdims=True))
    pr /= pr.sum(-1, keepdims=True)
    topi = np.argsort(-pr, axis=-1, kind="stable")[:, :TOPK]
    topw = np.take_along_axis(pr, topi, axis=-1)
    topw /= topw.sum(-1, keepdims=True)
    tok_lists = [np.where((topi == e).any(-1))[0] for e in range(E)]
    wts = [np.sum(np.where(topi[tl] == e, topw[tl], 0.0), -1).astype(np.float32)
           for e, tl in zip(range(E), tok_lists)]
    cap = max(128, -(-max(len(t) for t in tok_lists) // 64) * 64)
    assert cap <= 512, cap

    ck = (cap, DEBUG_TAPS)
    if ck not in _CACHE:
        _CACHE[ck] = _build(cap)
    nc = _CACHE[ck]

    inv_freq = 1.0 / (THETA ** (np.arange(0, HD, 2, dtype=np.float32) / HD))
    ang = positions[:, None] * inv_freq
    cos_t, sin_t = np.cos(ang), np.sin(ang)
    ident = np.eye(BLK, dtype=np.float32)

    # packed weights (shared across cores)
    wqkv_f = _pack_bf(wqkv * ln_in_w[:, None], SW)
    wo_p = _pack_bf(wo, SW)
    w13_p = _pack_bf(_interleave_cols(res_w13 * ln_res_w[:, None], H), SW)
    w2_p = _pack_bf(0.5 * res_w2, SW)
    wsT = ws.transpose(0, 2, 1)  # [E, H, 2I]
    wsT_il = [_interleave_cols(wsT[e], I) for e in range(E)]
    w2sT = w2s.transpose(0, 2, 1)  # [E, I, H]
    ws_pk = np.stack([_pack_dr(wsT_il[e], MSW) for e in range(E)])
    w2s_pk = np.stack([_pack_dr(0.5 * w2sT[e], MSW) for e in range(E)])

    x_norm_post = x_norm * ln_post_w

    shared = {
        "ident": h16(ident),
        "wqkv": wqkv_f, "wo": wo_p, "w13": w13_p, "w2": w2_p,
    }

    in_maps = []
    own = [[i, NBLK - 1 - i] for i in range(NC_)]
    for i in range(NC_):
        toks = np.concatenate([np.arange(b * BLK, (b + 1) * BLK) for b in own[i]])
        xnT = x_norm[toks].T          # [H, 256] normalized
        xrT = hidden[toks].T          # raw residual
        cs = np.tile(cos_t[toks].T, (1, 2))  # [64, 512]: duplicated per chunk pair
        sn = np.tile(sin_t[toks].T, (1, 2))
        # mask [key_p, slot, q]: slot=(c,sub) holds key block kb = c or 15-c
        mask = np.zeros((BLK, NBLK, TPC), np.float32)
        for c in range(NC_):
            for sub in range(2):
                kb = c if sub == 0 else NBLK - 1 - c
                kpos = np.arange(kb * BLK, (kb + 1) * BLK)
                mask[:, 2 * c + sub, :] = (toks[None, :] >= kpos[:, None])
        exps = [2 * i, 2 * i + 1]
        xg = np.zeros((EPC, H, cap), np.float32)
        ew = np.zeros((EPC, BLK, cap), np.float32)
        for j, e in enumerate(exps):
            n = len(tok_lists[e])
            xg[j, :, :n] = x_norm_post[tok_lists[e]].T
            ew[j, :, :n] = wts[e][None, :]
        in_maps.append({
            "xnb": h16(xnT.reshape(KQ, BLK, TPC).transpose(1, 0, 2).reshape(BLK, KQ * TPC)),
            "xr": h16(xrT.reshape(KQ, BLK, TPC).transpose(1, 0, 2).reshape(BLK, KQ * TPC)),
            "cos2": f32(cs), "sin2": f32(sn),
            "mask": f8(mask.reshape(BLK, NBLK * TPC)),
            "wsT": ws_pk[exps], "w2sT": w2s_pk[exps],
            "xgT": np.stack([f8(xg[j].reshape(KQ, BLK, cap).transpose(1, 0, 2)
                                .reshape(BLK, KQ * cap)) for j in range(EPC)]),
            "ew": bf(ew),
            **shared,
        })

    res = run_bass_kernel_spmd(nc, in_maps, core_ids=list(range(NC_)), trace=TRACE)
    LAST_RESULT = res

    out = np.zeros((T, H), np.float32)
    for i in range(NC_):
        toks = np.concatenate([np.arange(b * BLK, (b + 1) * BLK) for b in own[i]])
        ro = res.results[i]["res_out"].reshape(BLK, KQ, TPC).transpose(1, 0, 2) \
            .reshape(H, TPC)
        out[toks] = ro.T
    for i in range(NC_):
        for j, e in enumerate((2 * i, 2 * i + 1)):
            tl = tok_lists[e]
            mo = res.results[i]["moe_out"][j].reshape(BLK, KQ, cap) \
                .transpose(1, 0, 2).reshape(H, cap)
            out[tl] += mo.T[:len(tl)]
    return out


# revision 39
# speedup vs baseline: 1.0192x; 1.0192x over previous
"""ArcticDecoderLayer on 8 TRN2 NeuronCores — v2.

Sharding (token-parallel + expert-parallel):
  - tokens: zigzag blocks (core i owns 128-token blocks {i, 15-i});
    attention / wo / residual MLP token-parallel, weights replicated.
  - MoE: expert-parallel (2 experts/core), host-side routing + gather.
  - One fp8 KV AllGather.

v2 design:
  - MoE expert GEMMs in fp8e4 DoubleRow (256-deep contraction per matmul);
    qkv/wo/w13/w2 in fp16 (same PE speed as bf16, 4x less rounding --
    attention scores amplify operand rounding by ~sqrt(256)).
  - All weights host-prepacked to the exact SBUF stream layout (one
    contiguous run per partition per DMA; ~10x fewer descriptors).
  - Attention: head-pairs share the kv lhsT (uniform N=512 score matmuls,
    16 slots); batched Exp ACTs (N=1024, no bias); ALL causal masking
    (block + diagonal tri) via one resident 0/1 mask multiply; denominator
    from the SAME fp16 pdn as PV so quantization cancels in the ratio
    (one N=512 matmul per slot, fast-approx reciprocal).
  - silu via tanh (same ACT table set as Exp -> no table thrash);
    0.5 factor folded into w2/w2s weights host-side.
  - KV AllGather ships fp16 K+V as raw bytes; unpack lands in 2KB-run
    descriptors; MoE sweeps are thunks interleaved into the attention
    and w13 phases to keep the PE warm.
"""
import numpy as np
import ml_dtypes

import concourse.bacc as bacc
import concourse.tile as tile
import concourse.mybir as mybir
from concourse.bass_utils import run_bass_kernel_spmd

F32 = mybir.dt.float32
BF16 = mybir.dt.bfloat16
H16 = mybir.dt.float16
F8 = mybir.dt.float8e4
AF = mybir.ActivationFunctionType
DR = mybir.MatmulPerfMode.DoubleRow

H = 2048
NH = 16
NKV = 4
HD = 128
HALF = 64
I = 1024
E = 16
TOPK = 2
T = 2048
EPS = 1e-5
THETA = 10000.0
NC_ = 8
BLK = 128
NBLK = 16
TPC = 256  # tokens per core
EPC = 2  # experts per core
SCALE = HD ** -0.5
KQ = H // BLK  # 16
KK = KQ // 2  # 8 contraction pairs
MI = I // BLK  # 8

TRACE = False
DEBUG_TAPS = False
LAST_RESULT = None
_CACHE = {}

bf = lambda a: np.ascontiguousarray(np.asarray(a).astype(ml_dtypes.bfloat16))
h16 = lambda a: np.ascontiguousarray(np.asarray(a).astype(np.float16))
f32 = lambda a: np.ascontiguousarray(a, dtype=np.float32)
f8 = lambda a: np.ascontiguousarray(
    np.clip(np.asarray(a, dtype=np.float32), -240, 240).astype(ml_dtypes.float8_e4m3))

KG2 = 4  # contraction-pair chunks per weight-stream DMA
SW = 8   # m-chunks per sweep for TN=256 GEMMs
MSW = 2  # m-chunks per sweep for MoE GEMMs


def _pack_dr(W, sweep):
    """W [K, M] -> fp8 [128, (M//(128*sweep)) * (K//256) * sweep * 2 * 128]
    laid out [p, s, kk, jl, two, m] so any (sweep, kk-range) slice is one
    contiguous run per partition."""
    K, M = W.shape
    k2 = K // 256
    S = M // (BLK * sweep)
    W6 = W.reshape(k2, 2, BLK, S, sweep, BLK)      # kk, a, p, s, jl, m
    W6 = W6.transpose(2, 3, 0, 4, 1, 5)            # p, s, kk, jl, a, m
    return f8(W6.reshape(BLK, -1))


def _pack_bf(W, sweep):
    """W [K, M] -> bf16 [128, (M//(128*sweep)) * (K//128) * sweep * 128]
    laid out [p, s, k, jl, m]."""
    K, M = W.shape
    kc = K // BLK
    S = M // (BLK * sweep)
    W5 = W.reshape(kc, BLK, S, sweep, BLK)         # k, p, s, jl, m
    W5 = W5.transpose(1, 2, 0, 3, 4)               # p, s, k, jl, m
    return h16(W5.reshape(BLK, -1))


def _interleave_cols(w, half):
    # [rows, 2*half] -> column chunks reordered so chunk 2p=g_p, 2p+1=u_p
    rows = w.shape[0]
    g = w[:, :half].reshape(rows, half // BLK, BLK)
    u = w[:, half:].reshape(rows, half // BLK, BLK)
    out = np.empty((rows, 2 * (half // BLK), BLK), w.dtype)
    out[:, 0::2] = g
    out[:, 1::2] = u
    return out.reshape(rows, 2 * half // BLK * BLK)


def _build(cap):
    nc = bacc.Bacc("TRN2", target_bir_lowering=False, debug=False, num_devices=NC_)
    capk = cap // 2  # not used for contraction; cap is token width

    din = lambda name, shape, dt=F8: nc.dram_tensor(name, shape, dt, kind="ExternalInput")
    xnb_d = din("xnb", [BLK, KQ * TPC], H16)      # normalized x^T (qkv rhs), bf16
    xr_d = din("xr", [BLK, KQ * TPC], H16)        # raw x^T (residual), bf16
    cos_d = din("cos2", [HALF, 2 * TPC], F32)
    sin_d = din("sin2", [HALF, 2 * TPC], F32)
    mask_d = din("mask", [BLK, NBLK * TPC])  # [key_p, slot, q] 0/1 (incl. tri)
    ident_d = din("ident", [BLK, BLK], H16)
    wqkv_d = din("wqkv", [BLK, 3 * KQ * SW * BLK], H16)    # bf16 packed, 24 m-chunks
    wo_d = din("wo", [BLK, 2 * KQ * SW * BLK], H16)        # bf16, 16 m-chunks
    w13_d = din("w13", [BLK, 4 * KQ * SW * BLK], H16)       # 32 m-chunks (g/u interleaved)
    w2_d = din("w2", [BLK, 2 * KQ * SW * BLK], H16)         # 16 m-chunks
    ws_d = din("wsT", [EPC, BLK, 8 * KK * MSW * 2 * BLK])   # per expert 16 m-chunks
    w2s_d = din("w2sT", [EPC, BLK, 8 * (KK // 2) * MSW * 2 * BLK])  # 16 m, contraction I
    xg_d = din("xgT", [EPC, BLK, KQ * cap])
    ew_d = din("ew", [EPC, BLK, cap], BF16)

    res_out_d = nc.dram_tensor("res_out", [BLK, KQ * TPC], F32, kind="ExternalOutput")
    moe_out_d = nc.dram_tensor("moe_out", [EPC, BLK, KQ * cap], F32, kind="ExternalOutput")
    taps = {}
    if DEBUG_TAPS:
        for nm, w, dt in [("q", NH * TPC, H16), ("k", NKV * TPC, H16),
                          ("v8", 2 * NKV * HD, H16), ("kag", NC_ * NKV * 2 * BLK, H16),
                          ("vag", NC_ * 2 * NKV * BLK, H16),
                          ("pdn0", 2 * NBLK * TPC, H16), ("attn8", NH * TPC, H16),
                          ("resid", KQ * TPC, H16), ("h2", KQ * TPC, H16),
                          ("gu", KQ * TPC, H16), ("hm", EPC * MI * cap, F8),
                          ("apv0", 2 * TPC, F32), ("den0", 2 * TPC, F32)]:
            taps[nm] = nc.dram_tensor("tap_" + nm, [BLK, w], dt, kind="ExternalOutput")

    with tile.TileContext(nc) as tc:
        with (
            tc.tile_pool(name="res", bufs=1) as res,
            tc.tile_pool(name="stream", bufs=1 if DEBUG_TAPS else 2) as stream,
            tc.tile_pool(name="small", bufs=2) as small,
            tc.tile_pool(name="outp", bufs=2) as outp,
            tc.tile_pool(name="sps", bufs=2, space="PSUM") as sps,    # 2x [128,1024] = 4 banks
            tc.tile_pool(name="apv", bufs=1, space="PSUM") as apvp,   # 1 bank
            tc.tile_pool(name="dpsp", bufs=1, space="PSUM") as dpsp,  # 1 bank
            tc.tile_pool(name="macc", bufs=2, space="PSUM") as macc,  # 2 banks
            tc.tile_pool(name="dram", bufs=1, space="DRAM") as dram,
        ):
            eng_rr = [nc.sync, nc.scalar]

            # ---------------- resident loads ----------------
            xnb_sb = res.tile([BLK, KQ * TPC], H16, tag="xnb")
            nc.sync.dma_start(xnb_sb[:], xnb_d[:])
            cos2_sb = res.tile([HALF, 2 * TPC], F32, tag="cos")
            sin2_sb = res.tile([HALF, 2 * TPC], F32, tag="sin")
            nc.sync.dma_start(cos2_sb[:], cos_d[:])
            nc.sync.dma_start(sin2_sb[:], sin_d[:])
            ident_sb = res.tile([BLK, BLK], H16, tag="ident")
            nc.sync.dma_start(ident_sb[:], ident_d[:])
            mask_sb = res.tile([BLK, NBLK * TPC], F8, tag="mask")
            nc.scalar.dma_start(mask_sb[:], mask_d[:])
            xg_sb = res.tile([BLK, EPC * KQ * cap], F8, tag="xg")
            ew_sb = res.tile([BLK, EPC * cap], BF16, tag="ew")
            for e in range(EPC):
                nc.scalar.dma_start(xg_sb[:, e * KQ * cap:(e + 1) * KQ * cap], xg_d[e])
                nc.scalar.dma_start(ew_sb[:, e * cap:(e + 1) * cap], ew_d[e])
            xr_sb = res.tile([BLK, KQ * TPC], H16, tag="xr")
            nc.scalar.dma_start(xr_sb[:], xr_d[:])

            ones8_sb = res.tile([BLK, 32], F8, tag="ones8")
            nc.vector.memset(ones8_sb[:], 1.0)
            ones_row = res.tile([1, BLK], H16, tag="onesr")
            nc.vector.memset(ones_row[:], 1.0)
            ones_cb = res.tile([BLK, 1], H16, tag="onescb")
            nc.vector.memset(ones_cb[:], 1.0)

            q_sb = res.tile([BLK, NH * TPC], H16, tag="q")
            k8_sb = res.tile([BLK, NKV * TPC], H16, tag="k8")
            v8_sb = res.tile([BLK, 2 * NKV * HD], H16, tag="v8")
            kag_sb = res.tile([BLK, NC_ * NKV * 2 * BLK], H16, tag="kag")
            vag_sb = res.tile([BLK, NC_ * 2 * NKV * BLK], H16, tag="vag")
            attn8_sb = res.tile([BLK, NH * TPC], H16, tag="attn8")
            resid_sb = res.tile([BLK, KQ * TPC], H16, tag="resid")
            h2_sb = res.tile([BLK, KQ * TPC], H16, tag="h2")
            gu_sb = res.tile([BLK, KQ * TPC], H16, tag="gu")
            hm_sb = res.tile([BLK, EPC * MI * cap], F8, tag="hm")

            # ============ streamed GEMM sweep ============
            # dr=True: fp8 DoubleRow, weights [p, s, kk, jl, two, m],
            #   rhs_fn(kk) -> [128, 2, tn]; kkcnt = contraction pairs.
            # dr=False: bf16, weights [p, s, k, jl, m], rhs_fn(k) -> [128, tn];
            #   kkcnt = contraction chunks.
            def gemm(w_flat, mcnt, kkcnt, rhs_fn, tn, consume, sweep, tag,
                     kg2=KG2, sweep_starts=None, tile_w=1024, dr=True):
                nsweep = mcnt // sweep
                starts = sweep_starts if sweep_starts is not None else range(nsweep)
                per_tile = tile_w // tn  # m-chunks per psum tile
                wstep = (2 if dr else 1) * BLK  # weight cols per (k, j)
                for s in starts:
                    ntile = (sweep + per_tile - 1) // per_tile
                    if tag == "macct":
                        pts = [macc.tile([BLK, tile_w], F32, tag=tag, name=f"pt{j}")
                               for j in range(ntile)]
                    else:
                        pts = [sps.tile([BLK, tile_w], F32, tag=tag, name=f"pt{j}")
                               for j in range(ntile)]
                    paps = [pts[j // per_tile][:, (j % per_tile) * tn:(j % per_tile + 1) * tn]
                            for j in range(sweep)]
                    for kg0 in range(0, kkcnt, kg2):
                        kgn = min(kg2, kkcnt - kg0)
                        wt = stream.tile([BLK, KG2 * SW * (2 if dr else 1) * BLK],
                                         F8 if dr else H16, tag="wt")
                        off = (s * kkcnt + kg0) * sweep * wstep
                        eng_rr[(kg0 // kg2) % 2].dma_start(
                            wt[:, :kgn * sweep * wstep],
                            w_flat[:, off: off + kgn * sweep * wstep])
                        for kl in range(kgn):
                            kk = kg0 + kl
                            for j in range(sweep):
                                # bank = 512 f32; chunks sharing a bank pair their
                                # start/stop (start clears the whole bank)
                                per_bank = min(per_tile, max(1, 512 // tn))
                                jb = (j % per_tile) % per_bank
                                first = jb == 0
                                last = (jb == per_bank - 1) or (j == sweep - 1)
                                wap = wt[:, (kl * sweep + j) * wstep:(kl * sweep + j + 1) * wstep]
                                nc.tensor.matmul(
                                    paps[j],
                                    wap.rearrange("p (a m) -> p a m", a=2) if dr else wap,
                                    rhs_fn(kk),
                                    start=(kk == 0 and first),
                                    stop=(kk == kkcnt - 1 and last),
                                    perf_mode=DR if dr else None)
                    consume(s, sweep, paps, pts)

            # ---------------- QKV projection ----------------
            def rope_pair(pt2, dst, col0):
                # pt2 [128, 512] psum (two head-chunks side by side)
                t1 = small.tile([HALF, 2 * TPC], F32, tag="r1")
                t2 = small.tile([HALF, 2 * TPC], F32, tag="r2")
                nc.vector.tensor_mul(t1[:], pt2[0:HALF, :], cos2_sb[:])
                nc.vector.tensor_mul(t2[:], pt2[HALF:BLK, :], sin2_sb[:])
                nc.vector.tensor_sub(dst[0:HALF, col0:col0 + 2 * TPC], t1[:], t2[:])
                t3 = small.tile([HALF, 2 * TPC], F32, tag="r1")
                t4 = small.tile([HALF, 2 * TPC], F32, tag="r2")
                nc.vector.tensor_mul(t3[:], pt2[HALF:BLK, :], cos2_sb[:])
                nc.vector.tensor_mul(t4[:], pt2[0:HALF, :], sin2_sb[:])
                nc.vector.tensor_add(dst[HALF:BLK, col0:col0 + 2 * TPC], t3[:], t4[:])

            def qkv_consume(s, sweep, paps, pts):
                for jt, pt in enumerate(pts):
                    for half_t in range(2):
                        m = s * SW + jt * 4 + half_t * 2
                        pt2 = pt[:, half_t * 512:(half_t + 1) * 512]
                        if m < NH:
                            rope_pair(pt2, q_sb, m * TPC)
                        elif m < NH + NKV:
                            rope_pair(pt2, k8_sb, (m - NH) * TPC)
                        else:
                            for hj in range(2):
                                kvh = m + hj - NH - NKV
                                ps = pt[:, (half_t * 2 + hj) * TPC:(half_t * 2 + hj + 1) * TPC]
                                vtmp = small.tile([BLK, TPC], H16, tag="vtmp")
                                nc.vector.tensor_copy(vtmp[:], ps)
                                for tb in range(2):
                                    ptt = macc.tile([BLK, 1024], H16, tag="macct")
                                    nc.tensor.transpose(ptt[:, 0:BLK], vtmp[:, tb * BLK:(tb + 1) * BLK], ident_sb[:])
                                    nc.vector.tensor_copy(
                                        v8_sb[:, (tb * NKV + kvh) * BLK:(tb * NKV + kvh + 1) * BLK],
                                        ptt[:, 0:BLK])

            qkv_rhs = lambda k: xnb_sb[:, k * TPC:(k + 1) * TPC]
            # KV sweep first so the AllGather can launch early
            gemm(wqkv_d, 3 * SW, KQ, qkv_rhs, TPC, qkv_consume, SW, "acct",
                 sweep_starts=[2], dr=False)

            # ---------------- KV AllGather (K fp8 + V bf16, raw bytes) ----------------
            KSZ = NKV * BLK * TPC  # K bytes; V is 2*KSZ bytes
            U8 = mybir.dt.uint8
            kv_local = dram.tile([4 * KSZ], U8)
            kv_ag = dram.tile([NC_, 4 * KSZ], U8, addr_space="Shared")
            # K bf16: [d, (h sub t)] flat copy
            nc.sync.dma_start(kv_local[0:2 * KSZ].rearrange("(d x) -> d x", d=BLK),
                              k8_sb[:].bitcast(U8))
            # V bf16: [t, (sub h d)] flat copy
            nc.sync.dma_start(kv_local[2 * KSZ:4 * KSZ].rearrange("(t x) -> t x", t=BLK),
                              v8_sb[:].bitcast(U8))
            nc.gpsimd.collective_compute(
                "AllGather", mybir.AluOpType.bypass,
                replica_groups=[list(range(NC_))],
                ins=[kv_local[:]], outs=[kv_ag[:]])
            # remaining qkv sweeps (q heads) overlap the collective
            gemm(wqkv_d, 3 * SW, KQ, qkv_rhs, TPC, qkv_consume, SW, "acct",
                 sweep_starts=[0, 1], dr=False)
            # unpack: kag [d, (c h sub t)], vag [t, (c sub h d)]
            nc.sync.dma_start(
                kag_sb[:].bitcast(U8).rearrange("d (c x) -> d c x", c=NC_),
                kv_ag[:, 0:2 * KSZ].rearrange("c (d x) -> d c x", d=BLK))
            nc.scalar.dma_start(
                vag_sb[:].bitcast(U8).rearrange("t (c x) -> t c x", c=NC_),
                kv_ag[:, 2 * KSZ:4 * KSZ].rearrange("c (t x) -> t c x", t=BLK))

            # ------- MoE sweeps (thunks) interleaved with attention -------
            moe_thunks = []
            for e in range(EPC):
                gu_rhs = lambda kk, e=e: xg_sb[:, (e * KQ + 2 * kk) * cap:(e * KQ + 2 * kk + 2) * cap] \
                    .rearrange("p (a t) -> p a t", a=2)

                def gu_consume(s, sweep, paps, pts, e=e):
                    # sweep=2: chunks (2s, 2s+1) = (g_p, u_p) pair, p = s
                    gps, ups = paps[0], paps[1]
                    sg = small.tile([BLK, cap], BF16, tag="sg")
                    nc.scalar.activation(sg[:], gps, AF.Tanh, scale=0.5)
                    u8 = small.tile([BLK, cap], BF16, tag="u8")
                    nc.vector.tensor_copy(u8[:], ups)
                    t1 = small.tile([BLK, cap], BF16, tag="sgt1")
                    nc.vector.tensor_mul(t1[:], gps, u8[:])
                    t2 = small.tile([BLK, cap], BF16, tag="sgt2")
                    nc.vector.tensor_mul(t2[:], t1[:], sg[:])
                    nc.vector.tensor_add(
                        hm_sb[:, (e * MI + s) * cap:(e * MI + s + 1) * cap],
                        t1[:], t2[:])

                def w2s_consume(s, sweep, paps, pts, e=e):
                    for jl, ps in enumerate(paps):
                        m = s * MSW + jl
                        mo = outp.tile([BLK, cap], F32, tag="mo")
                        nc.vector.tensor_mul(mo[:], ps, ew_sb[:, e * cap:(e + 1) * cap])
                        nc.scalar.dma_start(
                            moe_out_d[e, :, m * cap:(m + 1) * cap], mo[:])

                w2s_rhs = lambda kk, e=e: hm_sb[:, (e * MI + 2 * kk) * cap:(e * MI + 2 * kk + 2) * cap] \
                    .rearrange("p (a t) -> p a t", a=2)

                for s in range(MI):  # 8 gu sweeps (one g/u pair each)
                    moe_thunks.append(lambda s=s, e=e, r=gu_rhs, c=gu_consume: gemm(
                        ws_d[e], 2 * MI, KK, r, cap, c, MSW, "macct",
                        sweep_starts=[s], tile_w=cap))
                for s in range(KQ // MSW):  # 8 w2s sweeps
                    moe_thunks.append(lambda s=s, e=e, r=w2s_rhs, c=w2s_consume: gemm(
                        w2s_d[e], KQ, KK // 2, r, cap, c, MSW, "macct",
                        sweep_starts=[s], tile_w=cap))

            # ---------------- attention (head pairs) ----------------
            def attention_scores(g):
                h0 = 2 * g
                qv = q_sb[:, h0 * TPC:(h0 + 2) * TPC]  # [d, 512]
                kvh = h0 // (NH // NKV)
                pdn = small.tile([BLK, 2 * NBLK * TPC], H16, tag="pdn",
                                 name=f"pdn{g}")  # [k, h2, slot, q] for PV
                for so in range(0, NBLK, 2):  # slot pairs -> one psum tile
                    spt = sps.tile([BLK, 1024], F32, tag="acct", name="spt")
                    for sl in (so, so + 1):
                        c, sub = sl // 2, sl % 2
                        kap = kag_sb[:, ((c * NKV + kvh) * 2 + sub) * BLK:
                                     ((c * NKV + kvh) * 2 + sub + 1) * BLK]
                        nc.tensor.matmul(spt[:, (sl - so) * 512:(sl - so + 1) * 512],
                                         kap, qv, start=True, stop=True)
                    # exp straight into pdn [k, h2, slot, q] (out AP in (s,h,q) order)
                    nc.scalar.activation(
                        pdn[:].rearrange("p (h s q) -> p s h q", h=2, s=NBLK)[:, so:so + 2, :, :],
                        spt[:], AF.Exp, scale=SCALE)
                    # mask in place
                    for hh in range(2):
                        nc.vector.tensor_mul(
                            pdn[:].rearrange("p (h s q) -> p h s q", h=2, s=NBLK)[:, hh, so:so + 2, :],
                            pdn[:].rearrange("p (h s q) -> p h s q", h=2, s=NBLK)[:, hh, so:so + 2, :],
                            mask_sb[:].rearrange("p (s q) -> p s q", s=NBLK)[:, so:so + 2, :])
                return pdn

            def attention_pv(g, pdn):
                h0 = 2 * g
                kvh = h0 // (NH // NKV)
                apv = apvp.tile([BLK, 2 * TPC], F32, tag="apvt")
                dps = dpsp.tile([BLK, 2 * TPC], F32, tag="dpst")
                for sl in range(NBLK):
                    c, sub = sl // 2, sl % 2
                    vap = vag_sb[:, ((c * 2 + sub) * NKV + kvh) * BLK:
                                 ((c * 2 + sub) * NKV + kvh + 1) * BLK]
                    nc.tensor.matmul(
                        apv[:], vap,
                        pdn[:].rearrange("p (h s q) -> p h s q", h=2, s=NBLK)[:, :, sl, :],
                        start=(sl == 0), stop=(sl == NBLK - 1))
                for sl in range(NBLK):
                    # den from the SAME fp16 pdn as PV so quantization cancels
                    # in the ratio; one MM covers both heads (free = (h, q)).
                    nc.tensor.matmul(
                        dps[0:1, :],
                        ones_cb[:],
                        pdn[:].rearrange("p (h s q) -> p h s q", h=2, s=NBLK)[:, :, sl, :],
                        start=(sl == 0), stop=(sl == NBLK - 1))
                if DEBUG_TAPS and g == 0:
                    at = small.tile([BLK, 2 * TPC], F32, tag="apvtap")
                    nc.vector.tensor_copy(at[:], apv[:])
                    nc.sync.dma_start(taps["apv0"].ap()[0:BLK, :], at[:])
                    dt_ = small.tile([1, 2 * TPC], F32, tag="dentap")
                    nc.vector.tensor_copy(dt_[:], dps[0:1, :])
                    nc.sync.dma_start(taps["den0"].ap()[0:1, :], dt_[:])
                # normalize: rec -> broadcast -> attn8
                rec32 = small.tile([1, 2 * TPC], F32, tag="rec32")
                nc.vector.reciprocal_approx_fast(rec32[:], dps[0:1, :])
                rec = small.tile([1, 2 * TPC], H16, tag="rec")
                nc.vector.tensor_copy(rec[:], rec32[:])
                bct = sps.tile([BLK, 1024], F32, tag="acct", name="bct")
                nc.tensor.matmul(bct[:, 0:512], ones_row[:], rec[:], start=True, stop=True)
                bcs = small.tile([BLK, 2 * TPC], H16, tag="bcs")
                nc.vector.tensor_copy(bcs[:], bct[:, 0:512])
                nc.vector.tensor_mul(attn8_sb[:, h0 * TPC:(h0 + 2) * TPC], apv[:], bcs[:])

            # front-load MoE sweeps to cover the AllGather; rest interleave.
            # scores(g) issue before pv(g-1) so the PV never waits on exp/mask.
            nfront, ntail = 3, 5
            for th in moe_thunks[:nfront]:
                th()
            rest = moe_thunks[nfront:len(moe_thunks) - ntail]
            tail_thunks = moe_thunks[len(moe_thunks) - ntail:]
            # scores(g+1) issue before pv(g): PV never waits on exp/mask
            ri = 0
            pdn_prev = attention_scores(0)
            if DEBUG_TAPS:
                nc.sync.dma_start(taps["pdn0"].ap(), pdn_prev[:])
            for g in range(1, NH // 2 + 1):
                for _ in range(3):
                    if ri < len(rest):
                        rest[ri]()
                        ri += 1
                if g < NH // 2:
                    pdn_cur = attention_scores(g)
                attention_pv(g - 1, pdn_prev)
                if g < NH // 2:
                    pdn_prev = pdn_cur
            while ri < len(rest):
                rest[ri]()
                ri += 1

            # ---------------- wo + residual ----------------
            def wo_consume(s, sweep, paps, pts):
                for jt, pt in enumerate(pts):
                    m0 = s * SW + jt * 4
                    for q in range(2):
                        nc.vector.tensor_add(
                            resid_sb[:, (m0 + 2 * q) * TPC:(m0 + 2 * q + 2) * TPC],
                            pt[:, q * 512:(q + 1) * 512],
                            xr_sb[:, (m0 + 2 * q) * TPC:(m0 + 2 * q + 2) * TPC])

            wo_rhs = lambda k: attn8_sb[:, k * TPC:(k + 1) * TPC]
            gemm(wo_d, 2 * SW, KQ, wo_rhs, TPC, wo_consume, SW, "acct", dr=False)

            # ---------------- residual MLP norm scale ----------------
            ssq = apvp.tile([BLK, 2 * TPC], F32, tag="apvt")
            for k in range(KQ):
                sq = small.tile([BLK, TPC], H16, tag="sq")
                nc.vector.tensor_mul(sq[:], resid_sb[:, k * TPC:(k + 1) * TPC],
                                     resid_sb[:, k * TPC:(k + 1) * TPC])
                nc.tensor.matmul(ssq[0:1, 0:TPC], ones_cb[:], sq[:],
                                 start=(k == 0), stop=(k == KQ - 1))
            vt = small.tile([1, TPC], F32, tag="vt")
            nc.vector.tensor_scalar(vt[:], ssq[0:1, 0:TPC], 1.0 / H, EPS,
                                    mybir.AluOpType.mult, mybir.AluOpType.add)
            st = small.tile([1, TPC], F32, tag="vt2")
            nc.scalar.activation(st[:], vt[:], AF.Sqrt)
            sr = small.tile([1, TPC], H16, tag="vt3")
            with nc.allow_low_precision(reason="rmsnorm rsqrt in bf16"):
                nc.vector.reciprocal(sr[:], st[:])
            s2p = dpsp.tile([BLK, 2 * TPC], F32, tag="dpst")
            nc.tensor.matmul(s2p[:, 0:TPC], ones_row[:], sr[:], start=True, stop=True)
            s2s = small.tile([BLK, TPC], F32, tag="s2s")
            nc.vector.tensor_copy(s2s[:], s2p[:, 0:TPC])
            for k in range(KQ):
                nc.vector.tensor_mul(h2_sb[:, k * TPC:(k + 1) * TPC],
                                     resid_sb[:, k * TPC:(k + 1) * TPC], s2s[:])

            # ---------------- w13 (interleaved g/u) + silu ----------------
            def w13_consume(s, sweep, paps, pts):
                for jt, pt in enumerate(pts):
                    for half_t in range(2):
                        p = (s * SW + jt * 4) // 2 + half_t
                        gps = pt[:, half_t * 512:half_t * 512 + TPC]
                        ups = pt[:, half_t * 512 + TPC:(half_t + 1) * 512]
                        sg = small.tile([BLK, TPC], BF16, tag="sg13")
                        nc.scalar.activation(sg[:], gps, AF.Tanh, scale=0.5)
                        u8 = small.tile([BLK, TPC], BF16, tag="u813")
                        nc.vector.tensor_copy(u8[:], ups)
                        t1 = small.tile([BLK, TPC], BF16, tag="t113")
                        nc.vector.tensor_mul(t1[:], gps, u8[:])
                        t2 = small.tile([BLK, TPC], BF16, tag="t213")
                        nc.vector.tensor_mul(t2[:], t1[:], sg[:])
                        nc.vector.tensor_add(gu_sb[:, p * TPC:(p + 1) * TPC],
                                             t1[:], t2[:])

            w13_rhs = lambda k: h2_sb[:, k * TPC:(k + 1) * TPC]
            for i, s in enumerate(range(4)):
                if i < len(tail_thunks):
                    tail_thunks[i]()
                gemm(w13_d, 4 * SW, KQ, w13_rhs, TPC, w13_consume, SW, "acct",
                     sweep_starts=[s], dr=False)
            for i in range(4, len(tail_thunks)):
                tail_thunks[i]()

            # ---------------- w2 + final out ----------------
            def w2_consume(s, sweep, paps, pts):
                for jt, pt in enumerate(pts):
                    m0 = s * SW + jt * 4
                    for q in range(2):
                        fo = outp.tile([BLK, 512], F32, tag="fo")
                        nc.vector.tensor_add(
                            fo[:], pt[:, q * 512:(q + 1) * 512],
                            resid_sb[:, (m0 + 2 * q) * TPC:(m0 + 2 * q + 2) * TPC])
                        nc.sync.dma_start(
                            res_out_d[:, (m0 + 2 * q) * TPC:(m0 + 2 * q + 2) * TPC],
                            fo[:])

            w2_rhs = lambda k: gu_sb[:, k * TPC:(k + 1) * TPC]
            gemm(w2_d, 2 * SW, KQ, w2_rhs, TPC, w2_consume, SW, "acct", dr=False)

            if DEBUG_TAPS:
                for nm, sb in [("q", q_sb), ("k", k8_sb), ("v8", v8_sb),
                               ("kag", kag_sb), ("vag", vag_sb),
                               ("attn8", attn8_sb), ("resid", resid_sb),
                               ("h2", h2_sb), ("gu", gu_sb), ("hm", hm_sb)]:
                    nc.sync.dma_start(taps[nm].ap(), sb[:])

    nc.compile()
    return nc


def kernel(**inputs):
    global LAST_RESULT
    hidden = f32(inputs["hidden_states"])
    positions = np.asarray(inputs["positions"]).astype(np.float32)
    ln_in_w = f32(inputs["ln_in_w"])
    ln_post_w = f32(inputs["ln_post_w"])
    ln_res_w = f32(inputs["ln_res_w"])
    wqkv = f32(inputs["wqkv"])
    wo = f32(inputs["wo"])
    res_w13 = f32(inputs["res_w13"])
    res_w2 = f32(inputs["res_w2"])
    gate_w = f32(inputs["gate_w"])
    ws = f32(inputs["ws"])
    w2s = f32(inputs["w2s"])

    # ---- host prep (routing + sharding) ----
    s = 1.0 / np.sqrt(np.mean(hidden * hidden, axis=1) + EPS)  # [T]
    x_norm = hidden * s[:, None]

    logits = (x_norm * ln_post_w) @ gate_w
    pr = np.exp(logits - logits.max(-1, keepdims=True))
    pr /= pr.sum(-1, keepdims=True)
    topi = np.argsort(-pr, axis=-1, kind="stable")[:, :TOPK]
    topw = np.take_along_axis(pr, topi, axis=-1)
    topw /= topw.sum(-1, keepdims=True)
    tok_lists = [np.where((topi == e).any(-1))[0] for e in range(E)]
    wts = [np.sum(np.where(topi[tl] == e, topw[tl], 0.0), -1).astype(np.float32)
           for e, tl in zip(range(E), tok_lists)]
    cap = max(128, -(-max(len(t) for t in tok_lists) // 64) * 64)
    assert cap <= 512, cap

    ck = (cap, DEBUG_TAPS)
    if ck not in _CACHE:
        _CACHE[ck] = _build(cap)
    nc = _CACHE[ck]

    inv_freq = 1.0 / (THETA ** (np.arange(0, HD, 2, dtype=np.float32) / HD))
    ang = positions[:, None] * inv_freq
    cos_t, sin_t = np.cos(ang), np.sin(ang)
    ident = np.eye(BLK, dtype=np.float32)

    # packed weights (shared across cores)
    wqkv_f = _pack_bf(wqkv * ln_in_w[:, None], SW)
    wo_p = _pack_bf(wo, SW)
    w13_p = _pack_bf(_interleave_cols(res_w13 * ln_res_w[:, None], H), SW)
    w2_p = _pack_bf(0.5 * res_w2, SW)
    wsT = ws.transpose(0, 2, 1)  # [E, H, 2I]
    wsT_il = [_interleave_cols(wsT[e], I) for e in range(E)]
    w2sT = w2s.transpose(0, 2, 1)  # [E, I, H]
    ws_pk = np.stack([_pack_dr(wsT_il[e], MSW) for e in range(E)])
    w2s_pk = np.stack([_pack_dr(0.5 * w2sT[e], MSW) for e in range(E)])

    x_norm_post = x_norm * ln_post_w

    shared = {
        "ident": h16(ident),
        "wqkv": wqkv_f, "wo": wo_p, "w13": w13_p, "w2": w2_p,
    }

    in_maps = []
    own = [[i, NBLK - 1 - i] for i in range(NC_)]
    for i in range(NC_):
        toks = np.concatenate([np.arange(b * BLK, (b + 1) * BLK) for b in own[i]])
        xnT = x_norm[toks].T          # [H, 256] normalized
        xrT = hidden[toks].T          # raw residual
        cs = np.tile(cos_t[toks].T, (1, 2))  # [64, 512]: duplicated per chunk pair
        sn = np.tile(sin_t[toks].T, (1, 2))
        # mask [key_p, slot, q]: slot=(c,sub) holds key block kb = c or 15-c
        mask = np.zeros((BLK, NBLK, TPC), np.float32)
        for c in range(NC_):
            for sub in range(2):
                kb = c if sub == 0 else NBLK - 1 - c
                kpos = np.arange(kb * BLK, (kb + 1) * BLK)
                mask[:, 2 * c + sub, :] = (toks[None, :] >= kpos[:, None])
        exps = [2 * i, 2 * i + 1]
        xg = np.zeros((EPC, H, cap), np.float32)
        ew = np.zeros((EPC, BLK, cap), np.float32)
        for j, e in enumerate(exps):
            n = len(tok_lists[e])
            xg[j, :, :n] = x_norm_post[tok_lists[e]].T
            ew[j, :, :n] = wts[e][None, :]
        in_maps.append({
            "xnb": h16(xnT.reshape(KQ, BLK, TPC).transpose(1, 0, 2).reshape(BLK, KQ * TPC)),
            "xr": h16(xrT.reshape(KQ, BLK, TPC).transpose(1, 0, 2).reshape(BLK, KQ * TPC)),
            "cos2": f32(cs), "sin2": f32(sn),
            "mask": f8(mask.reshape(BLK, NBLK * TPC)),
            "wsT": ws_pk[exps], "w2sT": w2s_pk[exps],
            "xgT": np.stack([f8(xg[j].reshape(KQ, BLK, cap).transpose(1, 0, 2)
                                .reshape(BLK, KQ * cap)) for j in range(EPC)]),
            "ew": bf(ew),
            **shared,
        })

    res = run_bass_kernel_spmd(nc, in_maps, core_ids=list(range(NC_)), trace=TRACE)
    LAST_RESULT = res

    out = np.zeros((T, H), np.float32)
    for i in range(NC_):
        toks = np.concatenate([np.arange(b * BLK, (b + 1) * BLK) for b in own[i]])
        ro = res.results[i]["res_out"].reshape(BLK, KQ, TPC).transpose(1, 0, 2) \
            .reshape(H, TPC)
        out[toks] = ro.T
    for i in range(NC_):
        for j, e in enumerate((2 * i, 2 * i + 1)):
            tl = tok_lists[e]
            mo = res.results[i]["moe_out"][j].reshape(BLK, KQ, cap) \
                .transpose(1, 0, 2).reshape(H, cap)
            out[tl] += mo.T[:len(tl)]
    return out
